# revision 7
# baseline (speedup 1.0000x reference)
"""BoundaryLoss Trainium2 kernel.

Computes mean((pred_boundary*w - target_boundary*w)^2) where boundaries are
|conv3d(x, sobel)| of argmax-class / target volumes, w = 3 where target in
SMALL_CLASSES else 1.

Sharding: data-parallel over 8 cores = 2 batches x 4 depth-chunks of 32
slices (+1 halo slice each side, host zero-padded). Each core returns
per-(partition, group) partial sums of 576*(pb-tb)^2*w^2; host does the
mean (the scalar all-reduce across shards).

Host-side layout (pure sharding prep): per-core logits transposed to
[H, C, DH, W] so each per-class chunk DMA is a 2-dim access pattern with
4KB contiguous runs per partition (~323GB/s vs ~280 for 512B runs);
target packed f16 (0..10 exact) as [H, DH, W], landing directly in the
padded persistent tile.

Device math (exact in f16/f32 integer arithmetic):
  conv3d(x, K) = (32*x - S_d S_h S_w x) / 24,  S = [1,2,1] separable
  A := 32*pred - S(pred) = 24*pred_boundary_signed  (ptP holds 32*pred)
  B := 32*t - S(t)                                  (psB holds B/32)
  loss partial = sum(((|A| - |B|) * w)^2) = 576 * sum((pb-tb)^2 w^2)
Pred side: the full separable 3x3 (d,w)-tap grid is folded into 9 matmuls
per PSUM group (weights -a_i*a_j*T/32, center I - T/8), so no S_w pass
exists on DVE for the pred volume and the PE taps read ptP directly
after the argmax extract. Target side: S_w as two shifted f16 adds on
DVE (head-of-chunk, gated only by the target DMA) + 4 matmul taps.

Argmax via custom DVE ops (registered in dve_ops at import): key_c =
fl(v_c + 1124) - 1024 + m*2^-17 with m = 10 - c. The f32 rounding at
magnitude ~1124 (ulp 2^-13) quantizes v+100 to a 16-ulp grid, so after
recentering to [90,110) (single binade, ulp 2^-17) the low 4 mantissa
bits hold m exactly; ties resolve to the smallest class like jnp.argmax.
AMAX_PAIRB folds bias+recenter+eps+max into ONE 1x DVE pass per tree
node (epsL = 2*epsR costs no constant slot; classes paired with m ratio
2:1: (10,5),(8,4),(6,3),(2,1)); classes with m=9,7 take an ACT bias pass
+ the 3-constant AMAX_PAIR; m=0 joins via AMAX_TAILB. 10 passes total.
idx = 10 - (key & 15); pred*32 = 320 - 32*(key & 15).

Scheduling (engines are strictly in-order): target DMA first per chunk
and all target-dependent DVE work (S_wT, weight map) ahead of the tree;
class DMAs in tree-consumption order; group post-processing (DVE
sub/mult + ACT square) emitted one chunk late so DVE never stalls on the
PE->ACT PSUM round-trip; out-DMA on the ACT queue so its end-of-rep wait
cannot block next-rep logits DMAs at SP's queue head. Pool deliberately
idle (slow in-order Q7; every dependent-work offload regressed).

Measured ~91-95us/rep steady-state (session-dependent) vs 141.5us
baseline; DMA floor ~79us.
"""

import numpy as np

B, C, D, H, W = 2, 11, 128, 128, 128
N_CORES = 8
DSH = 32            # output depth slices per shard
DH = DSH + 2        # input slices incl. halo
CHUNKS = (8, 8, 8, 6, 4)
N_GROUPS = DSH // 4  # 8 PSUM groups of 4 output slices

_CACHE = {}
_OPS = {}


def _register_custom_ops():
    """Register the fused argmax custom-DVE ops (documented extension point:
    dve_ops.OPS + _SUB_OPCODE_FOR_NAME + CUSTOM_DVE_SPECS). sha pins are
    computed from lower() so they can never drift."""
    global _OPS
    if _OPS:
        return _OPS
    from concourse import dve_ops
    from concourse.dve_spec import Spec, Src0, Src1, C0, C1, C2, maxx, lower
    from concourse.dve_uop import DveOpSpec

    specs = {
        # max((k1L - 1024) + epsL, (k1R - 1024) + epsR) on pre-biased inputs
        "AMAX_PAIR_ANT": Spec(
            body=maxx((Src0 - C0) + C1, (Src1 - C0) + C2),
            reference=lambda in0, in1, s0, s1, imm2: np.maximum(
                (in0 - s0) + s1, (in1 - s0) + np.float32(imm2)),
        ),
        # bias-folded pair: max(fl(v0+1124)-1024 + 2*eps, fl(v1+1124)-1024 + eps)
        "AMAX_PAIRB_ANT": Spec(
            body=maxx(((Src0 + C0) - C1) + (C2 + C2), ((Src1 + C0) - C1) + C2),
            reference=lambda in0, in1, s0, s1, imm2: np.maximum(
                ((in0 + s0) - s1) + np.float32(2 * imm2),
                ((in1 + s0) - s1) + np.float32(imm2)),
        ),
        # max(centered, fl(v+1124)-1024)  (eps = 0 leaf)
        "AMAX_TAILB_ANT": Spec(
            body=maxx(Src0, (Src1 + C0) - C1),
            reference=lambda in0, in1, s0, s1, imm2: np.maximum(
                in0, (in1 + s0) - s1),
        ),
    }
    for name, spec in specs.items():
        if name in dve_ops._SUB_OPCODE_FOR_NAME:
            _OPS[name] = next(o for o in dve_ops.OPS if o.name == name)
            continue
        row = max(dve_ops._SUB_OPCODE_FOR_NAME.values()) + 1
        assert row < 0x20, "custom-DVE row overflow"
        shas = {}
        for ver in ("v3", "v4"):
            try:
                u = lower(spec, ver=ver)
                shas[ver] = DveOpSpec(name=name, opcode=row, uops=u,
                                      rd1_en=True).sha(ver)
            except Exception:
                pass
        op = dve_ops.DveOp(name, spec, subdim=False, uops_sha=shas)
        dve_ops.OPS.append(op)
        dve_ops._SUB_OPCODE_FOR_NAME[name] = row
        dve_ops.CUSTOM_DVE_SPECS[name] = spec
        _OPS[name] = op
    return _OPS


def _group_schedule(chunks, n_groups):
    """groups emitted after each chunk: group g needs input slices <= 4g+5."""
    sched, done = [], 0
    end = 0
    for nd in chunks:
        end += nd
        gs = []
        while done < n_groups and 4 * done + 5 <= end - 1:
            gs.append(done)
            done += 1
        sched.append(gs)
    assert done == n_groups, (sched, done)
    return sched


def _make_wmats():
    """[4,128,128] f16: I, -T/32, -T/16, I-T/8 with T = tridiag(1,2,1)."""
    T = np.zeros((128, 128), np.float32)
    i = np.arange(128)
    T[i, i] = 2.0
    T[i[:-1], i[:-1] + 1] = 1.0
    T[i[:-1] + 1, i[:-1]] = 1.0
    I = np.eye(128, dtype=np.float32)
    wm = np.stack([I, -T / 32.0, -T / 16.0, I - T / 8.0])
    return wm.astype(np.float16)


EPS = [(10 - c) * 2.0 ** -17 for c in range(C)]


def _build_nc(dh, chunks, reps=1, dyn_reps=False, stage="full", taps9=True,
              extract_dve=False, lg_bufs=None, wk_bufs=4,
              taps9b=False,
              dma_order=(0, 5, 2, 6, 1, 3, 4, 7, 8, 9, 10), ps_bufs=4):
    # stage: "dma" | "argmax" | "smooth" | "full" — prefix subsets for
    # bottleneck isolation (timing experiments only; grading uses "full").
    import concourse.bacc as bacc
    import concourse.mybir as mybir
    from concourse.tile import TileContext

    ops = _register_custom_ops()
    PAIR = ops["AMAX_PAIR_ANT"]
    PAIRB, TAILB = ops["AMAX_PAIRB_ANT"], ops["AMAX_TAILB_ANT"]

    f32, f16, i32 = mybir.dt.float32, mybir.dt.float16, mybir.dt.int32
    A = mybir.AluOpType
    AF = mybir.ActivationFunctionType

    dsh = dh - 2
    n_groups = dsh // 4
    sched = _group_schedule(chunks, n_groups)
    max_nd = max(chunks)

    nc = bacc.Bacc()
    lg = nc.declare_dram_parameter("logits", [H, C, dh, W], f32, isOutput=False)
    tg = nc.declare_dram_parameter("target", [H, dh, W], f16, isOutput=False)
    wm = nc.declare_dram_parameter("wmats", [4, 128, 128], f16, isOutput=False)
    out = nc.declare_dram_parameter("out", [128, n_groups], f32, isOutput=True)
    nrp = (nc.declare_dram_parameter("nreps", [1, 1], i32, isOutput=False)
           if dyn_reps else None)

    PW = 130  # width padded with a zero column each side

    with TileContext(nc) as tc:
        from contextlib import ExitStack

        with ExitStack() as ctx:
            cpool = ctx.enter_context(tc.tile_pool(name="const", bufs=1))
            lgpool = ctx.enter_context(tc.tile_pool(name="lg", bufs=lg_bufs or (2 * C + 4)))
            pers = ctx.enter_context(tc.tile_pool(name="pers", bufs=1))
            swpool = ctx.enter_context(tc.tile_pool(name="sw", bufs=4))
            wkpool = ctx.enter_context(tc.tile_pool(name="wk", bufs=wk_bufs))
            uvpool = ctx.enter_context(tc.tile_pool(name="uv", bufs=6))
            pspool = ctx.enter_context(tc.tile_pool(name="ps", bufs=ps_bufs, space="PSUM"))

            # constants
            wt = cpool.tile([128, 4, 128], f16, tag="wt")
            nc.sync.dma_start(out=wt[:, :, :], in_=wm[:, :, :].rearrange("k h m -> h k m"))
            W_I, W_T1, W_T2, W_C = (wt[:, 0, :], wt[:, 1, :], wt[:, 2, :],
                                    wt[:, 3, :])

            # persistent volumes (halo-resident in SBUF)
            ptP = pers.tile([128, dh, PW], f16, tag="ptP")   # 32*pred, w-padded
            ptT = pers.tile([128, dh, PW], f16, tag="ptT")   # target,  w-padded
            xswP = pers.tile([128, dh, 128], f16, tag="xswP")
            xswT = pers.tile([128, dh, 128], f16, tag="xswT")
            wmap = pers.tile([128, dh, 128], f16, tag="wmap")
            acc = pers.tile([128, n_groups], f32, tag="acc")

            # zero padded buffers once; interiors rewritten per chunk
            nc.scalar.memzero(ptP[:, :, :])
            nc.scalar.memzero(ptT[:, :, :])
            nc.vector.memset(acc[:, :], 0.0)

            uv_of = {}

            def emit_group_mm(g):
                """matmul taps + PSUM-draining abs; all psA taps first so
                abs(psA) overlaps the psB matmuls."""
                psA = pspool.tile([128, 512], f32, tag="ps")
                psB = pspool.tile([128, 512], f32, tag="ps")
                if taps9:
                    # S_w folded into the taps: psA = sum_ij V_ij ptP[d+i,w+j],
                    # V = -a_i a_j T/32 (+I at center), a = [1,2,1]
                    P = ptP
                    nc.tensor.matmul(psA[:, :], W_C, P[:, 4*g+1 : 4*g+5, 1:129],
                                     start=True, stop=False)
                    for sl in (P[:, 4*g+1 : 4*g+5, 0:128], P[:, 4*g+1 : 4*g+5, 2:130],
                               P[:, 4*g : 4*g+4, 1:129], P[:, 4*g+2 : 4*g+6, 1:129]):
                        nc.tensor.matmul(psA[:, :], W_T2, sl, start=False, stop=False)
                    corners = (P[:, 4*g : 4*g+4, 0:128], P[:, 4*g : 4*g+4, 2:130],
                               P[:, 4*g+2 : 4*g+6, 0:128], P[:, 4*g+2 : 4*g+6, 2:130])
                    for i, sl in enumerate(corners):
                        nc.tensor.matmul(psA[:, :], W_T1, sl, start=False,
                                         stop=(i == 3))
                else:
                    nc.tensor.matmul(psA[:, :], W_I, ptP[:, 4 * g + 1 : 4 * g + 5, 1:129],
                                     start=True, stop=False)
                    nc.tensor.matmul(psA[:, :], W_T1, xswP[:, 4 * g : 4 * g + 4, :],
                                     start=False, stop=False)
                    nc.tensor.matmul(psA[:, :], W_T1, xswP[:, 4 * g + 2 : 4 * g + 6, :],
                                     start=False, stop=False)
                    nc.tensor.matmul(psA[:, :], W_T2, xswP[:, 4 * g + 1 : 4 * g + 5, :],
                                     start=False, stop=True)
                if taps9b:
                    Q = ptT
                    nc.tensor.matmul(psB[:, :], W_C, Q[:, 4*g+1 : 4*g+5, 1:129],
                                     start=True, stop=False)
                    for sl in (Q[:, 4*g+1 : 4*g+5, 0:128], Q[:, 4*g+1 : 4*g+5, 2:130],
                               Q[:, 4*g : 4*g+4, 1:129], Q[:, 4*g+2 : 4*g+6, 1:129]):
                        nc.tensor.matmul(psB[:, :], W_T2, sl, start=False, stop=False)
                    cornersB = (Q[:, 4*g : 4*g+4, 0:128], Q[:, 4*g : 4*g+4, 2:130],
                                Q[:, 4*g+2 : 4*g+6, 0:128], Q[:, 4*g+2 : 4*g+6, 2:130])
                    for i, sl in enumerate(cornersB):
                        nc.tensor.matmul(psB[:, :], W_T1, sl, start=False,
                                         stop=(i == 3))
                else:
                    nc.tensor.matmul(psB[:, :], W_I, ptT[:, 4 * g + 1 : 4 * g + 5, 1:129],
                                     start=True, stop=False)
                    nc.tensor.matmul(psB[:, :], W_T1, xswT[:, 4 * g : 4 * g + 4, :],
                                     start=False, stop=False)
                    nc.tensor.matmul(psB[:, :], W_T1, xswT[:, 4 * g + 2 : 4 * g + 6, :],
                                     start=False, stop=False)
                    nc.tensor.matmul(psB[:, :], W_T2, xswT[:, 4 * g + 1 : 4 * g + 5, :],
                                     start=False, stop=True)
                # |A|, |B| (B accumulated at 1/32 scale -> scale=32 on abs)
                u = uvpool.tile([128, 512], f16, tag="u")
                v = uvpool.tile([128, 512], f16, tag="v")
                nc.scalar.activation(u[:, :], psA[:, :], AF.Abs)
                nc.scalar.activation(v[:, :], psB[:, :], AF.Abs, scale=32.0)
                uv_of[g] = (u, v)

            def emit_group_post(g):
                """DVE sub/mult + ACT square, emitted one chunk late so the
                in-order DVE never stalls on the PE->ACT round-trip."""
                u, v = uv_of.pop(g)
                e = wkpool.tile([128, 512], f16, tag="e")
                ew = wkpool.tile([128, 512], f16, tag="ew")
                scr = wkpool.tile([128, 512], f32, tag="scr")
                nc.vector.tensor_tensor(e[:, :], u[:, :], v[:, :], A.subtract)
                nc.vector.tensor_tensor(ew[:, :], e[:, :],
                                        wmap[:, 4 * g + 1 : 4 * g + 5, :],
                                        A.mult)
                nc.scalar.activation(scr[:, :], ew[:, :], AF.Square,
                                     accum_out=acc[:, g : g + 1])

            if dyn_reps:
                nrt = cpool.tile([1, 1], i32, tag="nrt", name="nrt")
                nc.sync.dma_start(out=nrt[0:1, 0:1], in_=nrp[0:1, 0:1])
                regs = nc.alloc_registers("nreps_r")
                for eng_t, reg in zip(mybir.ALL_ENGINES, regs.handles):
                    nc.engines[eng_t].reg_load(reg, nrt[0:1, 0:1])
                rv = nc.snap(regs, donate=True, min_val=1, max_val=1 << 20)
                rep_cm = tc.For_i(0, rv, 1)
            else:
                rep_cm = tc.For_i(0, reps, 1) if reps > 1 else None
            if rep_cm is not None:
                rep_cm.__enter__()
            d0 = 0
            for ci, nd in enumerate(chunks):
                FD = nd * 128
                # target DMA first: the target-dependent DVE work (S_wT,
                # wa/wb/wd) runs at the head of the chunk while class DMAs
                # land; class DMAs in tree-consumption order
                nc.sync.dma_start(out=ptT[:, d0 : d0 + nd, 1:129],
                                  in_=tg[:, d0 : d0 + nd, :])
                lts = [None] * C
                for c in dma_order:
                    t = lgpool.tile([128, max_nd * 128], f32, tag="lg",
                                    name=f"lg{c}")
                    nc.sync.dma_start(
                        out=t[:, 0 : nd * 128],
                        in_=lg[:, c, d0 : d0 + nd, :].rearrange("h d w -> h (d w)"),
                    )
                    lts[c] = t


                def F(t):  # flat f32 view [128, FD]
                    return t[:, 0:FD]

                def V(t):  # [128, nd, 128] view
                    return t[:, 0:FD].rearrange("p (d w) -> p d w", w=128)

                def I(t):  # flat int32 view
                    return F(t).bitcast(i32)

                if stage == "dma":
                    for c in range(C):
                        nc.scalar.activation(acc[0:1, c % 8 : c % 8 + 1],
                                             lts[c][0:1, 0:1], AF.Copy)
                    nc.scalar.activation(acc[0:1, 0:1], ptT[0:1, d0, 1:2],
                                         AF.Copy)
                    d0 += nd
                    continue

                # weight map first: DVE fills the head-of-chunk window
                # where only the target has landed
                if not taps9b:
                    t1T = swpool.tile([128, max_nd, 129], f16, tag="t1T")
                    nc.vector.tensor_tensor(t1T[:, 0:nd, :],
                                            ptT[:, d0 : d0 + nd, 0:129],
                                            ptT[:, d0 : d0 + nd, 1:130], A.add)
                    nc.vector.tensor_tensor(xswT[:, d0 : d0 + nd, :],
                                            t1T[:, 0:nd, 0:128],
                                            t1T[:, 0:nd, 1:129], A.add)
                wa = wkpool.tile([128, max_nd, 128], f16, tag="wa")
                wb = wkpool.tile([128, max_nd, 128], f16, tag="wb")
                nc.vector.tensor_scalar(wa[:, 0:nd, :], ptT[:, d0 : d0 + nd, 1:129],
                                        2.0, -2.0, A.is_lt, A.mult)
                nc.vector.tensor_scalar(wb[:, 0:nd, :], ptT[:, d0 : d0 + nd, 1:129],
                                        4.0, -2.0, A.is_equal, A.mult)
                nc.vector.tensor_tensor(wmap[:, d0 : d0 + nd, :], wa[:, 0:nd, :],
                                        wb[:, 0:nd, :], A.add)
                nc.scalar.activation(wmap[:, d0 : d0 + nd, :],
                                     wmap[:, d0 : d0 + nd, :], AF.Copy, bias=3.0)

                # --- argmax: fused custom max tree (DVE), bias folded in ---
                # k = fl(v + 1124) - 1024 + m*2^-17, m = 10 - c: the f32
                # rounding at magnitude ~1124 (ulp 2^-13) quantizes v+100 to a
                # 16-ulp grid, so the low 4 mantissa bits carry m exactly.
                # PAIRB folds the bias and pairs classes with m ratio 2:1
                # (epsL = 2*imm2 costs no constant slot). Classes 1,3 (m=9,7)
                # go through an ACT bias + the 3-constant PAIR op.
                E = 2.0 ** -17
                for c in (1, 3):
                    nc.scalar.activation(F(lts[c]), F(lts[c]), AF.Copy,
                                         bias=1124.0)
                for a, b, mr in ((0, 5, 5), (2, 6, 4), (4, 7, 3), (8, 9, 1)):
                    nc.vector._custom_dve(PAIRB, out=F(lts[a]), in0=F(lts[a]),
                                          in1=F(lts[b]), s0=1124.0, s1=1024.0,
                                          imm2=mr * E)
                nc.vector._custom_dve(PAIR, out=F(lts[1]), in0=F(lts[1]),
                                      in1=F(lts[3]), s0=1024.0,
                                      s1=9 * E, imm2=7 * E)
                mx = nc.vector.tensor_tensor
                mx(F(lts[0]), F(lts[0]), F(lts[2]), A.max)
                mx(F(lts[4]), F(lts[4]), F(lts[8]), A.max)
                nc.vector._custom_dve(TAILB, out=F(lts[1]), in0=F(lts[1]),
                                      in1=F(lts[10]), s0=1124.0, s1=1024.0)
                mx(F(lts[0]), F(lts[0]), F(lts[4]), A.max)
                mx(F(lts[0]), F(lts[0]), F(lts[1]), A.max)
                # extract: jt = key & 15; pred*32 = 320 - 32*jt
                nc.vector.tensor_scalar(I(lts[1]), I(lts[0]), 15, None,
                                        A.bitwise_and)
                if extract_dve:
                    # all-DVE extract: PE taps wait only on DVE, no ACT hop
                    nc.vector.tensor_scalar(ptP[:, d0 : d0 + nd, 1:129],
                                            V(lts[1]).bitcast(i32), -32, 320,
                                            A.mult, A.add)
                else:
                    nc.scalar.activation(ptP[:, d0 : d0 + nd, 1:129],
                                         V(lts[1]).bitcast(i32),
                                         AF.Copy, scale=-32.0, bias=320.0)
                if stage == "argmax":
                    nc.scalar.activation(acc[0:1, 0:1], ptP[0:1, d0, 1:2],
                                         AF.Copy)
                    d0 += nd
                    continue

                # --- pred-side smoothing (skipped when folded into taps) ---
                if not taps9:
                  t1P = swpool.tile([128, max_nd, 129], f16, tag="t1P")
                  nc.vector.tensor_tensor(t1P[:, 0:nd, :], ptP[:, d0 : d0 + nd, 0:129],
                                        ptP[:, d0 : d0 + nd, 1:130], A.add)
                  nc.vector.tensor_tensor(xswP[:, d0 : d0 + nd, :],
                                        t1P[:, 0:nd, 0:128], t1P[:, 0:nd, 1:129],
                                        A.add)

                if stage == "smooth":
                    nc.scalar.activation(acc[0:1, 2:3], xswP[0:1, d0, 0:1],
                                         AF.Copy)
                    nc.scalar.activation(acc[0:1, 3:4], xswT[0:1, d0, 0:1],
                                         AF.Copy)
                    nc.scalar.activation(acc[0:1, 4:5], wmap[0:1, d0, 0:1],
                                         AF.Copy)
                else:
                    for g in sched[ci]:
                        emit_group_mm(g)
                    if ci > 0:
                        for g in sched[ci - 1]:
                            emit_group_post(g)
                d0 += nd
            if stage == "full":
                for g in sched[len(chunks) - 1]:
                    emit_group_post(g)
            if rep_cm is not None:
                rep_cm.__exit__(None, None, None)

            # out-DMA from the ACT queue: its end-of-rep wait must not sit
            # at the head of SP's queue blocking next-rep logits DMA issues
            nc.scalar.dma_start(out=out[:, :], in_=acc[:, :])
    nc.compile()
    return nc


def _get_built(dh=DH, chunks=CHUNKS):
    key = (dh, tuple(chunks))
    if key not in _CACHE:
        _CACHE[key] = _build_nc(dh, chunks)
    return _CACHE[key]


def _shard_inputs(logits, target):
    """FULL inputs -> 8 per-core in_maps (b-major, then depth chunk).
    Host work is sharding + layout transposes only."""
    lp = np.zeros((B, C, D + 2, H, W), np.float32)
    lp[:, :, 1:-1] = logits
    tp = np.zeros((B, D + 2, H, W), np.float16)
    tp[:, 1:-1] = target[:, 0]  # 0..10, exact in f16
    wm = _make_wmats()
    maps = []
    for b in range(B):
        for j in range(D // DSH):
            s = j * DSH
            maps.append({
                "logits": np.ascontiguousarray(
                    lp[b, :, s : s + DH].transpose(2, 0, 1, 3)),   # [H,C,DH,W]
                "target": np.ascontiguousarray(
                    tp[b, s : s + DH].transpose(1, 0, 2)),         # [H,DH,W]
                "wmats": wm,
            })
    return maps


def kernel(logits: np.ndarray, target: np.ndarray) -> np.ndarray:
    from concourse.bass_utils import run_bass_kernel_spmd

    nc = _get_built()
    maps = _shard_inputs(np.asarray(logits), np.asarray(target))
    res = run_bass_kernel_spmd(nc, maps, list(range(N_CORES))).results
    total = 0.0
    for r in res:
        total += np.asarray(r["out"], np.float64).sum()
    loss = total / (576.0 * B * D * H * W)
    return np.float32(loss)


# ---------------- numpy reference for one shard (testing only) ----------------

def shard_partial_np(lg, tgt):
    """lg [C,dh,H,W] f32 (already +halo, zero-padded), tgt [dh,H,W] i32.
    Returns sum over interior slices of 576*(pb-tb)^2*w^2."""
    pred = np.argmax(lg, axis=0).astype(np.float32)
    t = tgt.astype(np.float32)

    def S(x):
        xp = np.pad(x, ((0, 0), (1, 1), (1, 1)))
        s = xp[:, :, :-2] + 2 * xp[:, :, 1:-1] + xp[:, :, 2:]
        s = s[:, :-2, :] + 2 * s[:, 1:-1, :] + s[:, 2:, :]
        return s[:-2] + 2 * s[1:-1] + s[2:]

    Av = 32 * pred[1:-1] - S(pred)
    Bv = 32 * t[1:-1] - S(t)
    w = np.where((tgt[1:-1] < 2) | (tgt[1:-1] == 4), 1.0, 3.0).astype(np.float32)
    e = (np.abs(Av) - np.abs(Bv)) * w
    return float(np.sum((e * e).astype(np.float64)))
